# revision 1
# baseline (speedup 1.0000x reference)
"""BrickTube kernel for 8x Trainium2 NeuronCores.

The reference "BrickTube" module applies 80 tiny (2,2,2,2) gate cores to a
[B, 1024] state tensor. Every gate application is linear in x and
INPUT_DIM == BINDIM == OUTPUT_DIM == 1024, so the whole module collapses to

    out = x @ W,   W[i, :] = circuit(e_i)  (1024 x 1024)

W is built exactly on the host in float64 from `cores` (cheap: 80 small
tensordots), then the device runs a batch-sharded dense matmul:
each of the 8 cores computes y_c^T = W^T @ x_c^T for its 4096-row shard of x,
with fp16 operands (same PE rate as bf16, 8x the mantissa accuracy) and fp32
PSUM accumulation.

Device kernel structure (per core):
  - j-outer loop over 8 batch chunks of 512 so input DMA delivery stays ahead
    of PE consumption (each j-chunk is 1MB of x^T vs ~14us of matmuls).
  - 8 PSUM banks (one per output-row chunk m), accumulated over the 8
    contraction chunks k; drained bank-by-bank while the next j streams.
  - ~3.5us of tiny warmup matmuls on zeros so the PE HAM clock-gate is at
    full rate (2.4 GHz) by the time real data lands.
  - x-chunk DMAs on the Sync HWDGE ring, w on the Scalar ring (parallel
    issue); PSUM drains alternate between Vector and Scalar engines.
"""

import math

import ml_dtypes
import numpy as np

# ---- problem constants (hardcoded per contract) ----
B = 32768
D = 1024
N_CORES = 8
NPC = B // N_CORES  # 4096 batch rows per core

BOND = 2
Q = 10
N_LAYERS = 8
PAIRS1 = [(i, i + 1) for i in range(0, Q, 2)]
PAIRS2 = [(i, (i + 1) % Q) for i in range(1, Q, 2)]
HALF = Q // 2


def build_w(cores: np.ndarray) -> np.ndarray:
    """Collapse the 80-gate circuit into W [1024, 1024] (float64),
    with out_row = x_row @ W."""
    c = cores.astype(np.float64)
    s = np.eye(D, dtype=np.float64).reshape((D,) + (BOND,) * Q)
    for layer in range(N_LAYERS):
        base = layer * Q
        for g, (i, j) in enumerate(PAIRS1):
            s = np.tensordot(s, c[base + g], axes=((i + 1, j + 1), (0, 1)))
            s = np.moveaxis(s, (-2, -1), (i + 1, j + 1))
        for g, (i, j) in enumerate(PAIRS2):
            s = np.tensordot(s, c[base + HALF + g], axes=((i + 1, j + 1), (0, 1)))
            s = np.moveaxis(s, (-2, -1), (i + 1, j + 1))
    return s.reshape(D, D)


_NC_CACHE = None


def _build_bass():
    """Device program (identical on all 8 cores):
      inputs:  xt [1024, 4096] fp16  (x-shard transposed: xt[k, n])
               w  [1024, 1024] fp16  (W[k, m], k = contraction)
      output:  yt [1024, 4096] fp32  (y-shard transposed: yt[m, n])
    yt[m, n] = sum_k w[k, m] * xt[k, n]
    """
    global _NC_CACHE
    if _NC_CACHE is not None:
        return _NC_CACHE

    import concourse.bacc as bacc
    import concourse.mybir as mybir
    import concourse.tile as tile

    KC = D // 128  # 8 contraction chunks
    MC = D // 128  # 8 output-row chunks
    JC = NPC // 512  # 8 batch column chunks
    F16 = mybir.dt.float16
    F32 = mybir.dt.float32

    nc = bacc.Bacc("TRN2")
    xt = nc.dram_tensor("xt", [D, NPC], F16, kind="ExternalInput")
    w = nc.dram_tensor("w", [D, D], F16, kind="ExternalInput")
    yt = nc.dram_tensor("yt", [D, NPC], F32, kind="ExternalOutput")

    with tile.TileContext(nc) as tc:
        with (
            tc.tile_pool(name="xpool", bufs=1) as xpool,
            tc.tile_pool(name="wpool", bufs=1) as wpool,
            tc.tile_pool(name="opool", bufs=2) as opool,
            tc.tile_pool(name="psum", bufs=1, space="PSUM") as ppool,
        ):
            # ---- PE warmup: ~2.3us of tiny matmuls on zeros so HAM is at
            # K=8/8 by the time real data lands. Output goes to the ps7 slot
            # (the last bank the first real accumulation touches).
            warm = xpool.tile([128, 64], F16, name="warm", tag="warm")
            nc.gpsimd.memset(warm[:], 0)
            wps = ppool.tile([128, 64], F32, name="wps", tag="ps7")
            for _ in range(56):
                nc.tensor.matmul(wps[0:64, :], warm[:], warm[:])

            # ---- input loads: x j-chunks on the Sync ring (j=0 split four
            # ways so the first matmuls start sooner, rest in two), w chunks
            # on the Scalar ring.
            xj = []
            for j in range(JC):
                xtile = xpool.tile([128, KC * 512], F16, name=f"xj{j}", tag=f"x{j}")
                src = xt[:, j * 512 : (j + 1) * 512]
                pieces = 4 if j == 0 else (2 if j == 1 else 1)
                kk = KC // pieces  # k-chunks per piece
                for p in range(pieces):
                    nc.sync.dma_start(
                        xtile[
                            :, p * kk * 512 : (p + 1) * kk * 512
                        ].rearrange("p (k n) -> p k n", n=512),
                        src[p * kk * 128 : (p + 1) * kk * 128, :].rearrange(
                            "(k p) n -> p k n", p=128
                        ),
                    )
                xj.append(xtile)
            wk = []
            for k in range(KC):
                wt = wpool.tile([128, D], F16, name=f"wk{k}", tag=f"w{k}")
                nc.scalar.dma_start(wt[:], w[k * 128 : (k + 1) * 128, :])
                wk.append(wt)

            # ---- main loop: j outer, accumulate over k into 8 PSUM banks
            # (one per m), drain while the next j computes.
            for j in range(JC):
                psums = [
                    ppool.tile([128, 512], F32, name=f"ps{m}", tag=f"ps{m}")
                    for m in range(MC)
                ]
                for k in range(KC):
                    rhs = xj[j][:, k * 512 : (k + 1) * 512]
                    for m in range(MC):
                        nc.tensor.matmul(
                            psums[m][:],
                            wk[k][:, m * 128 : (m + 1) * 128],
                            rhs,
                            start=(k == 0),
                            stop=(k == KC - 1),
                        )
                # drain: copies split DVE/ACT by m parity; one DMA per m-pair
                # (4 triggers per j instead of 8 — trigger issue is ~600ns of
                # Sync occupancy each and binds the kernel tail).
                last_j = j == JC - 1
                for mp in range(MC // 2):
                    osb = opool.tile(
                        [128, 2 * 512], F32, name=f"osb{mp}", tag=f"osb{mp}"
                    )
                    nc.vector.tensor_copy(osb[:, :512], psums[2 * mp][:])
                    nc.scalar.copy(osb[:, 512:], psums[2 * mp + 1][:])
                    if last_j and mp == MC // 2 - 1:
                        # final pair as two singles: the kernel tail waits on
                        # the very last transfer, so keep it small (256KB)
                        nc.sync.dma_start(
                            yt[
                                2 * mp * 128 : (2 * mp + 1) * 128,
                                j * 512 : (j + 1) * 512,
                            ],
                            osb[:, :512],
                        )
                        nc.sync.dma_start(
                            yt[
                                (2 * mp + 1) * 128 : (2 * mp + 2) * 128,
                                j * 512 : (j + 1) * 512,
                            ],
                            osb[:, 512:],
                        )
                    else:
                        nc.sync.dma_start(
                            yt[
                                2 * mp * 128 : (2 * mp + 2) * 128,
                                j * 512 : (j + 1) * 512,
                            ].rearrange("(m p) n -> p m n", p=128),
                            osb[:].rearrange("p (m n) -> p m n", n=512),
                        )

    nc.compile()
    _NC_CACHE = nc
    return nc


def _run(x: np.ndarray, cores: np.ndarray, trace: bool = False, trace_cores=None):
    from concourse.bass_utils import run_bass_kernel_spmd

    W = build_w(cores)
    wb = np.ascontiguousarray(W.astype(np.float32).astype(np.float16))

    xb = x.astype(np.float16)
    in_maps = []
    for c in range(N_CORES):
        xt_c = np.ascontiguousarray(xb[c * NPC : (c + 1) * NPC, :].T)
        in_maps.append({"xt": xt_c, "w": wb})

    nc = _build_bass()
    kwargs = {}
    if trace_cores is not None:
        kwargs["trace_cores"] = trace_cores
    res = run_bass_kernel_spmd(
        nc, in_maps, core_ids=list(range(N_CORES)), trace=trace, **kwargs
    )

    y = np.empty((B, D), dtype=np.float32)
    for c in range(N_CORES):
        y[c * NPC : (c + 1) * NPC, :] = res.results[c]["yt"].T
    return y, res


def kernel(x: np.ndarray, cores: np.ndarray) -> np.ndarray:
    y, _ = _run(x, cores, trace=False)
    return y



# revision 2
# speedup vs baseline: 1.2488x; 1.2488x over previous
"""BrickTube kernel for 8x Trainium2 NeuronCores.

The reference "BrickTube" module applies 80 tiny (2,2,2,2) gate cores to a
[B, 1024] state tensor. Every gate application is linear in x and
INPUT_DIM == BINDIM == OUTPUT_DIM == 1024, so the whole module collapses to

    out = x @ W,   W[i, :] = circuit(e_i)  (1024 x 1024)

W is built exactly on the host in float64 from `cores`, then the device runs a
batch-sharded dense matmul: each of the 8 cores computes y_c^T = W^T @ x_c^T
for its 4096-row shard of x.

Mixed-precision split-K: W's row norms span ~200x. The 768 contraction rows
with the smallest ||W_row|| (holding ~15% of the output energy) are computed
in fp8 e4m3 with MatmulPerfMode.DoubleRow (2 K-subtiles per instruction at
0.5 PE cycles/col — 2x the fp16 MAC rate); the 256 largest rows stay fp16.
Host-measured rel err of this split vs float64: ~1.5e-2 (gate: 2e-2).

Scale folding: W8 = e4m3(W_sel8 * S), W16 = fp16(W_sel16 * S) with S a power
of two chosen so W8's absmax sits just under e4m3's 240 max-normal. All
matmuls then accumulate S*y into a single PSUM bank per output block, and the
PSUM->SBUF drain applies the exact 1/S scale for free (scaled copy). Output
is written fp16 (negligible extra error) and upcast on host.

Device kernel structure (per core, per 512-col batch chunk j):
  phase 1: 24 DoubleRow fp8 matmuls, k-pair-outer / m-inner so the first
           w8 DMA piece covers the first 8 matmuls;
  phase 2: per m: 2 fp16 matmuls finishing bank m, then an immediate scaled
           drain (DVE/ACT alternating by m parity) and a pair-DMA out —
           output flush overlaps the rest of the chunk's compute, so the
           kernel tail after the last matmul is only the last 128KB DMA.
Input x chunks stream on the GpSimd DMA ring, w on the Scalar ring, outputs
on the Sync ring. A short warmup matmul burst covers the PE HAM clock ramp
while the first x/w pieces are in flight.
"""

import math

import ml_dtypes
import numpy as np

# ---- problem constants (hardcoded per contract) ----
B = 32768
D = 1024
N_CORES = 8
NPC = B // N_CORES  # 4096 batch rows per core

BOND = 2
Q = 10
N_LAYERS = 8
PAIRS1 = [(i, i + 1) for i in range(0, Q, 2)]
PAIRS2 = [(i, (i + 1) % Q) for i in range(1, Q, 2)]
HALF = Q // 2

K8 = 768  # contraction rows computed in fp8 (smallest-norm rows of W)
K16 = D - K8  # rows kept in fp16
T8 = K8 // 128  # 6 fp8 k-subtiles -> 3 DoubleRow pairs
T16 = K16 // 128  # 2 fp16 k-subtiles
JC = NPC // 512  # 8 batch column chunks
MC = D // 128  # 8 output-row chunks


def build_w(cores: np.ndarray) -> np.ndarray:
    """Collapse the 80-gate circuit into W [1024, 1024] (float64),
    with out_row = x_row @ W."""
    c = cores.astype(np.float64)
    s = np.eye(D, dtype=np.float64).reshape((D,) + (BOND,) * Q)
    for layer in range(N_LAYERS):
        base = layer * Q
        for g, (i, j) in enumerate(PAIRS1):
            s = np.tensordot(s, c[base + g], axes=((i + 1, j + 1), (0, 1)))
            s = np.moveaxis(s, (-2, -1), (i + 1, j + 1))
        for g, (i, j) in enumerate(PAIRS2):
            s = np.tensordot(s, c[base + HALF + g], axes=((i + 1, j + 1), (0, 1)))
            s = np.moveaxis(s, (-2, -1), (i + 1, j + 1))
    return s.reshape(D, D)


_NC_CACHE = None


def _build_bass(inv_scale: float):
    """Device program (identical on all 8 cores):
      inputs:  x8  [768, 4096] e4m3 (x-shard cols sel8, transposed)
               x16 [256, 4096] fp16 (x-shard cols sel16, transposed)
               w8  [768, 1024] e4m3 (W[sel8]  * S, k-major)
               w16 [256, 1024] fp16 (W[sel16] * S)
      output:  yt  [1024, 4096] fp16 (y-shard transposed)
    yt[m, n] = inv_scale * (sum_k w8[k,m] x8[k,n] + sum_k w16[k,m] x16[k,n])
    """
    global _NC_CACHE
    if _NC_CACHE is not None:
        return _NC_CACHE

    import concourse.bacc as bacc
    import concourse.mybir as mybir
    import concourse.tile as tile

    F8 = mybir.dt.float8e4
    F16 = mybir.dt.float16
    F32 = mybir.dt.float32
    DR = mybir.MatmulPerfMode.DoubleRow

    nc = bacc.Bacc("TRN2")
    x8 = nc.dram_tensor("x8", [K8, NPC], F8, kind="ExternalInput")
    x16 = nc.dram_tensor("x16", [K16, NPC], F16, kind="ExternalInput")
    w8 = nc.dram_tensor("w8", [K8, D], F8, kind="ExternalInput")
    w16 = nc.dram_tensor("w16", [K16, D], F16, kind="ExternalInput")
    yt = nc.dram_tensor("yt", [D, NPC], F16, kind="ExternalOutput")

    with tile.TileContext(nc) as tc:
        with (
            tc.tile_pool(name="xpool", bufs=1) as xpool,
            tc.tile_pool(name="wpool", bufs=1) as wpool,
            tc.tile_pool(name="opool", bufs=2) as opool,
            tc.tile_pool(name="psum", bufs=1, space="PSUM") as ppool,
        ):
            # ---- PE warmup: tiny matmuls on zeros cover the HAM clock ramp
            # while the first x8/w8 DMA pieces are in flight.
            warm = xpool.tile([128, 64], F16, name="warm", tag="warm")
            nc.gpsimd.memset(warm[:], 0)
            wps = ppool.tile([128, 64], F32, name="wps", tag="ps7")
            for _ in range(10):
                nc.tensor.matmul(wps[0:64, :], warm[:], warm[:])

            # ---- weight loads on the Scalar ring: w8 in pair-pieces so the
            # first piece covers phase 1's first 8 matmuls, w16 afterwards
            # (not needed until phase 2).
            w8t = wpool.tile([128, T8 * D], F8, name="w8t", tag="w8t")
            for p in range(T8 // 2):
                nc.scalar.dma_start(
                    w8t[:, 2 * p * D : (2 * p + 2) * D].rearrange(
                        "p (t m) -> p t m", m=D
                    ),
                    w8[2 * p * 128 : (2 * p + 2) * 128, :].rearrange(
                        "(t p) m -> p t m", p=128
                    ),
                )
            w16t = wpool.tile([128, T16 * D], F16, name="w16t", tag="w16t")
            nc.scalar.dma_start(
                w16t[:].rearrange("p (t m) -> p t m", m=D),
                w16[:].rearrange("(t p) m -> p t m", p=128),
            )

            # ---- x chunk loads on the GpSimd ring, j-major (x8 then x16)
            x8j = []
            x16j = []
            for j in range(JC):
                t8 = xpool.tile([128, T8 * 512], F8, name=f"x8j{j}", tag=f"x8{j}")
                nc.gpsimd.dma_start(
                    t8[:].rearrange("p (t n) -> p t n", n=512),
                    x8[:, j * 512 : (j + 1) * 512].rearrange(
                        "(t p) n -> p t n", p=128
                    ),
                )
                x8j.append(t8)
                t16 = xpool.tile([128, T16 * 512], F16, name=f"x16j{j}", tag=f"x16{j}")
                nc.gpsimd.dma_start(
                    t16[:].rearrange("p (t n) -> p t n", n=512),
                    x16[:, j * 512 : (j + 1) * 512].rearrange(
                        "(t p) n -> p t n", p=128
                    ),
                )
                x16j.append(t16)

            # ---- main loop over batch chunks
            for j in range(JC):
                psums = [
                    ppool.tile([128, 512], F32, name=f"ps{m}", tag=f"ps{m}")
                    for m in range(MC)
                ]
                x8v = x8j[j][:].rearrange("p (t n) -> p t n", n=512)
                w8v = w8t[:].rearrange("p (t m) -> p t m", m=D)
                # phase 1: fp8 DoubleRow matmuls, k-pair-outer / m-inner
                for tp in range(T8 // 2):
                    for m in range(MC):
                        nc.tensor.matmul(
                            psums[m][:],
                            w8v[:, 2 * tp : 2 * tp + 2, m * 128 : (m + 1) * 128],
                            x8v[:, 2 * tp : 2 * tp + 2, :],
                            start=(tp == 0),
                            stop=False,
                            perf_mode=DR,
                        )
                # phase 2: fp16 matmuls finish each bank; drain + DMA chase
                last_j = j == JC - 1
                for m in range(MC):
                    for t in range(T16):
                        nc.tensor.matmul(
                            psums[m][:],
                            w16t[:, t * D + m * 128 : t * D + (m + 1) * 128],
                            x16j[j][:, t * 512 : (t + 1) * 512],
                            start=False,
                            stop=(t == T16 - 1),
                        )
                    mp = m // 2
                    if m % 2 == 0:
                        osb = opool.tile(
                            [128, 2 * 512], F16, name=f"osb{mp}", tag=f"osb{mp}"
                        )
                        nc.vector.tensor_scalar_mul(osb[:, :512], psums[m][:], inv_scale)
                    else:
                        nc.scalar.mul(osb[:, 512:], psums[m][:], inv_scale)
                        if last_j and mp == MC // 2 - 1:
                            # final pair as two singles: the kernel tail waits
                            # on the very last transfer, so keep it small
                            nc.sync.dma_start(
                                yt[
                                    2 * mp * 128 : (2 * mp + 1) * 128,
                                    j * 512 : (j + 1) * 512,
                                ],
                                osb[:, :512],
                            )
                            nc.sync.dma_start(
                                yt[
                                    (2 * mp + 1) * 128 : (2 * mp + 2) * 128,
                                    j * 512 : (j + 1) * 512,
                                ],
                                osb[:, 512:],
                            )
                        else:
                            nc.sync.dma_start(
                                yt[
                                    2 * mp * 128 : (2 * mp + 2) * 128,
                                    j * 512 : (j + 1) * 512,
                                ].rearrange("(m p) n -> p m n", p=128),
                                osb[:].rearrange("p (m n) -> p m n", n=512),
                            )

    nc.compile()
    _NC_CACHE = nc
    return nc


def _prepare(x: np.ndarray, cores: np.ndarray):
    """Host-side: build W, pick the fp8/fp16 row split, quantize operands."""
    W = build_w(cores)
    rn = np.sqrt((W * W).sum(axis=1))
    order = np.argsort(rn, kind="stable")
    sel8 = order[:K8]
    sel16 = order[K8:]

    amax8 = float(np.abs(W[sel8]).max())
    amax16 = float(np.abs(W[sel16]).max())
    # keep W8 under e4m3's 240 max-normal and W16*S comfortably inside fp16
    s_pow = min(
        math.floor(math.log2(216.0 / max(amax8, 1e-30))),
        math.floor(math.log2(30000.0 / max(amax16, 1e-30))),
    )
    S = float(2.0**s_pow)

    w8b = np.ascontiguousarray(
        (W[sel8] * S).astype(np.float32).astype(ml_dtypes.float8_e4m3)
    )
    w16b = np.ascontiguousarray((W[sel16] * S).astype(np.float32).astype(np.float16))

    x8_full = np.ascontiguousarray(
        x[:, sel8].astype(np.float32).astype(ml_dtypes.float8_e4m3).T
    )
    x16_full = np.ascontiguousarray(x[:, sel16].astype(np.float32).astype(np.float16).T)
    return w8b, w16b, x8_full, x16_full, 1.0 / S


def _run(x: np.ndarray, cores: np.ndarray, trace: bool = False, trace_cores=None):
    from concourse.bass_utils import run_bass_kernel_spmd

    w8b, w16b, x8_full, x16_full, inv_scale = _prepare(x, cores)

    in_maps = []
    for c in range(N_CORES):
        in_maps.append(
            {
                "x8": np.ascontiguousarray(x8_full[:, c * NPC : (c + 1) * NPC]),
                "x16": np.ascontiguousarray(x16_full[:, c * NPC : (c + 1) * NPC]),
                "w8": w8b,
                "w16": w16b,
            }
        )

    nc = _build_bass(inv_scale)
    kwargs = {}
    if trace_cores is not None:
        kwargs["trace_cores"] = trace_cores
    res = run_bass_kernel_spmd(
        nc, in_maps, core_ids=list(range(N_CORES)), trace=trace, **kwargs
    )

    y = np.empty((B, D), dtype=np.float32)
    for c in range(N_CORES):
        y[c * NPC : (c + 1) * NPC, :] = res.results[c]["yt"].T.astype(np.float32)
    return y, res


def kernel(x: np.ndarray, cores: np.ndarray) -> np.ndarray:
    y, _ = _run(x, cores, trace=False)
    return y


# revision 5
# speedup vs baseline: 1.5173x; 1.2150x over previous
"""BrickTube kernel for 8x Trainium2 NeuronCores.

The reference "BrickTube" module applies 80 tiny (2,2,2,2) gate cores to a
[B, 1024] state tensor. Every gate application is linear in x and
INPUT_DIM == BINDIM == OUTPUT_DIM == 1024, so the whole module collapses to

    out = x @ W,   W[i, :] = circuit(e_i)  (1024 x 1024)

W is built exactly on the host in float64 from `cores`, then the device runs a
batch-sharded dense matmul: each of the 8 cores computes y_c^T = W^T @ x_c^T
for its 4096-row shard of x.

Mixed-precision split-K: W's row norms span ~200x. The 768 contraction rows
with the smallest ||W_row|| (holding ~15% of the output energy) are computed
in fp8 e4m3 with MatmulPerfMode.DoubleRow (2 K-subtiles per instruction at
0.5 PE cycles/col — 2x the fp16 MAC rate); the 256 largest rows stay fp16.
Host-measured rel err of this split vs float64: ~1.5e-2 (gate: 2e-2).

Scale folding: W8 = e4m3(W_sel8 * S), W16 = fp16(W_sel16 * S) with S a power
of two chosen so W8's absmax sits just under e4m3's 240 max-normal. All
matmuls then accumulate S*y into a single PSUM bank per output block, and the
PSUM->SBUF drain applies the exact 1/S scale for free (scaled copy). Output
is written fp16 (negligible extra error) and upcast on host.

Device kernel structure (per core, per 512-col batch chunk j):
  phase 1: 24 DoubleRow fp8 matmuls, k-pair-outer / m-inner so the first
           w8 DMA piece covers the first 8 matmuls;
  phase 2: per m: 2 fp16 matmuls finishing bank m, then an immediate scaled
           drain (DVE/ACT alternating by m parity) and a pair-DMA out —
           output flush overlaps the rest of the chunk's compute, so the
           kernel tail after the last matmul is only the last 128KB DMA.
Input x chunks stream on the GpSimd DMA ring, w on the Scalar ring, outputs
on the Sync ring. A short warmup matmul burst covers the PE HAM clock ramp
while the first x/w pieces are in flight.
"""

import math

import ml_dtypes
import numpy as np

# ---- problem constants (hardcoded per contract) ----
B = 32768
D = 1024
N_CORES = 8
NPC = B // N_CORES  # 4096 batch rows per core

BOND = 2
Q = 10
N_LAYERS = 8
PAIRS1 = [(i, i + 1) for i in range(0, Q, 2)]
PAIRS2 = [(i, (i + 1) % Q) for i in range(1, Q, 2)]
HALF = Q // 2

K8 = 768  # contraction rows computed in fp8 (smallest-norm rows of W)
K16 = D - K8  # rows kept in fp16
T8 = K8 // 128  # 6 fp8 k-subtiles -> 3 DoubleRow pairs
T16 = K16 // 128  # 2 fp16 k-subtiles
JC = NPC // 512  # 8 batch column chunks
MC = D // 128  # 8 output-row chunks


def build_w(cores: np.ndarray) -> np.ndarray:
    """Collapse the 80-gate circuit into W [1024, 1024] (float64),
    with out_row = x_row @ W."""
    c = cores.astype(np.float64)
    s = np.eye(D, dtype=np.float64).reshape((D,) + (BOND,) * Q)
    for layer in range(N_LAYERS):
        base = layer * Q
        for g, (i, j) in enumerate(PAIRS1):
            s = np.tensordot(s, c[base + g], axes=((i + 1, j + 1), (0, 1)))
            s = np.moveaxis(s, (-2, -1), (i + 1, j + 1))
        for g, (i, j) in enumerate(PAIRS2):
            s = np.tensordot(s, c[base + HALF + g], axes=((i + 1, j + 1), (0, 1)))
            s = np.moveaxis(s, (-2, -1), (i + 1, j + 1))
    return s.reshape(D, D)


_NC_CACHE = None


def _build_bass(inv_scale: float):
    """Device program (identical on all 8 cores):
      inputs:  x8  [768, 4096] e4m3 (x-shard cols sel8, transposed)
               x16 [256, 4096] fp16 (x-shard cols sel16, transposed)
               w8  [768, 1024] e4m3 (W[sel8]  * S, k-major)
               w16 [256, 1024] fp16 (W[sel16] * S)
      output:  yt  [1024, 4096] fp16 (y-shard transposed)
    yt[m, n] = inv_scale * (sum_k w8[k,m] x8[k,n] + sum_k w16[k,m] x16[k,n])
    """
    global _NC_CACHE
    if _NC_CACHE is not None:
        return _NC_CACHE

    import concourse.bacc as bacc
    import concourse.mybir as mybir
    import concourse.tile as tile

    F8 = mybir.dt.float8e4
    F16 = mybir.dt.float16
    F32 = mybir.dt.float32
    DR = mybir.MatmulPerfMode.DoubleRow

    nc = bacc.Bacc("TRN2")
    x8 = nc.dram_tensor("x8", [K8, NPC], F8, kind="ExternalInput")
    x16 = nc.dram_tensor("x16", [K16, NPC], F16, kind="ExternalInput")
    w8 = nc.dram_tensor("w8", [K8, D], F8, kind="ExternalInput")
    w16 = nc.dram_tensor("w16", [K16, D], F16, kind="ExternalInput")
    yt = nc.dram_tensor("yt", [D, NPC], F16, kind="ExternalOutput")

    with tile.TileContext(nc) as tc:
        with (
            tc.tile_pool(name="xpool", bufs=1) as xpool,
            tc.tile_pool(name="wpool", bufs=1) as wpool,
            tc.tile_pool(name="opool", bufs=2) as opool,
            tc.tile_pool(name="psum", bufs=1, space="PSUM") as ppool,
        ):
            # ---- PE warmup: matmuls on zeros cover the HAM clock ramp
            # while the first x8/w8 DMA pieces are in flight (~2.5us).
            warm = xpool.tile([128, 512], F16, name="warm", tag="warm")
            nc.gpsimd.memset(warm[:], 0)
            wps = ppool.tile([128, 512], F32, name="wps", tag="ps7")
            for _ in range(5):
                nc.tensor.matmul(wps[0:128, :], warm[:, :128], warm[:])

            # ---- weight loads on the Scalar ring: w8 in pair-pieces so the
            # first piece covers phase 1's first 8 matmuls, w16 afterwards
            # (not needed until phase 2).
            w8t = wpool.tile([128, T8 * D], F8, name="w8t", tag="w8t")
            for p in range(T8 // 2):
                nc.scalar.dma_start(
                    w8t[:, 2 * p * D : (2 * p + 2) * D].rearrange(
                        "p (t m) -> p t m", m=D
                    ),
                    w8[2 * p * 128 : (2 * p + 2) * 128, :].rearrange(
                        "(t p) m -> p t m", p=128
                    ),
                )
            w16t = wpool.tile([128, T16 * D], F16, name="w16t", tag="w16t")
            nc.scalar.dma_start(
                w16t[:].rearrange("p (t m) -> p t m", m=D),
                w16[:].rearrange("(t p) m -> p t m", p=128),
            )

            # ---- x chunk loads. j=0 goes on the otherwise-idle Sync ring in
            # pieces (smallest first) so the first DoubleRow pair's data lands
            # ASAP; j=1..7 stream on the Scalar ring behind the w loads.
            x8j = []
            x16j = []
            for j in range(JC):
                t8 = xpool.tile([128, T8 * 512], F8, name=f"x8j{j}", tag=f"x8{j}")
                src8 = x8[:, j * 512 : (j + 1) * 512]
                if j == 0:
                    for lo, hi in ((0, 2), (2, 6)):
                        nc.sync.dma_start(
                            t8[:, lo * 512 : hi * 512].rearrange(
                                "p (t n) -> p t n", n=512
                            ),
                            src8[lo * 128 : hi * 128, :].rearrange(
                                "(t p) n -> p t n", p=128
                            ),
                        )
                else:
                    nc.scalar.dma_start(
                        t8[:].rearrange("p (t n) -> p t n", n=512),
                        src8.rearrange("(t p) n -> p t n", p=128),
                    )
                x8j.append(t8)
                t16 = xpool.tile([128, T16 * 512], F16, name=f"x16j{j}", tag=f"x16{j}")
                eng = nc.sync if j == 0 else nc.scalar
                eng.dma_start(
                    t16[:].rearrange("p (t n) -> p t n", n=512),
                    x16[:, j * 512 : (j + 1) * 512].rearrange(
                        "(t p) n -> p t n", p=128
                    ),
                )
                x16j.append(t16)

            # ---- main loop over batch chunks
            # j=0: k-pair-outer fp8 phase then fp16 phase, so the first w8/x8
            # DMA pieces cover the first 8 matmuls and w16's later arrival
            # doesn't stall the PE.
            # j>=1: m-outer "full-finish" — each bank's 5 matmuls then an
            # immediate drain + DMA, spreading the output flush across the
            # whole chunk (the kernel tail only waits on bank 7's 256KB).
            w8v = w8t[:].rearrange("p (t m) -> p t m", m=D)

            def dr_mm(psum, m, tp, j, start):
                x8v = x8j[j][:].rearrange("p (t n) -> p t n", n=512)
                nc.tensor.matmul(
                    psum[:],
                    w8v[:, 2 * tp : 2 * tp + 2, m * 128 : (m + 1) * 128],
                    x8v[:, 2 * tp : 2 * tp + 2, :],
                    start=start,
                    stop=False,
                    perf_mode=DR,
                )

            def f16_mm(psum, m, t, j):
                nc.tensor.matmul(
                    psum[:],
                    w16t[:, t * D + m * 128 : t * D + (m + 1) * 128],
                    x16j[j][:, t * 512 : (t + 1) * 512],
                    start=False,
                    stop=(t == T16 - 1),
                )

            def drain(psums, m, j, last_j):
                mp = m // 2
                if m % 2 == 0:
                    osb = opool.tile(
                        [128, 2 * 512], F16, name=f"osb{mp}", tag=f"osb{mp}"
                    )
                    nc.vector.tensor_scalar_mul(osb[:, :512], psums[m][:], inv_scale)
                    return osb
                nc.scalar.mul(osb_live[0][:, 512:], psums[m][:], inv_scale)
                osb = osb_live[0]
                if last_j and mp == MC // 2 - 1:
                    # final pair as two singles: the kernel tail waits on
                    # the very last transfer, so keep it small
                    nc.sync.dma_start(
                        yt[
                            2 * mp * 128 : (2 * mp + 1) * 128,
                            j * 512 : (j + 1) * 512,
                        ],
                        osb[:, :512],
                    )
                    nc.sync.dma_start(
                        yt[
                            (2 * mp + 1) * 128 : (2 * mp + 2) * 128,
                            j * 512 : (j + 1) * 512,
                        ],
                        osb[:, 512:],
                    )
                else:
                    nc.sync.dma_start(
                        yt[
                            2 * mp * 128 : (2 * mp + 2) * 128,
                            j * 512 : (j + 1) * 512,
                        ].rearrange("(m p) n -> p m n", p=128),
                        osb[:].rearrange("p (m n) -> p m n", n=512),
                    )
                return None

            osb_live = [None]
            for j in range(JC):
                psums = [
                    ppool.tile([128, 512], F32, name=f"ps{m}", tag=f"ps{m}")
                    for m in range(MC)
                ]
                last_j = j == JC - 1
                if j == 0:
                    for tp in range(T8 // 2):
                        for m in range(MC):
                            dr_mm(psums[m], m, tp, j, start=(tp == 0))
                    for m in range(MC):
                        for t in range(T16):
                            f16_mm(psums[m], m, t, j)
                        osb = drain(psums, m, j, last_j)
                        if osb is not None:
                            osb_live[0] = osb
                else:
                    for m in range(MC):
                        for tp in range(T8 // 2):
                            dr_mm(psums[m], m, tp, j, start=(tp == 0))
                        for t in range(T16):
                            f16_mm(psums[m], m, t, j)
                        osb = drain(psums, m, j, last_j)
                        if osb is not None:
                            osb_live[0] = osb

    nc.compile()
    _NC_CACHE = nc
    return nc


def _prepare(x: np.ndarray, cores: np.ndarray):
    """Host-side: build W, pick the fp8/fp16 row split, quantize operands."""
    W = build_w(cores)
    rn = np.sqrt((W * W).sum(axis=1))
    order = np.argsort(rn, kind="stable")
    sel8 = order[:K8]
    sel16 = order[K8:]

    amax8 = float(np.abs(W[sel8]).max())
    amax16 = float(np.abs(W[sel16]).max())
    # keep W8 under e4m3's 240 max-normal and W16*S comfortably inside fp16
    s_pow = min(
        math.floor(math.log2(216.0 / max(amax8, 1e-30))),
        math.floor(math.log2(30000.0 / max(amax16, 1e-30))),
    )
    S = float(2.0**s_pow)

    w8b = np.ascontiguousarray(
        (W[sel8] * S).astype(np.float32).astype(ml_dtypes.float8_e4m3)
    )
    w16b = np.ascontiguousarray((W[sel16] * S).astype(np.float32).astype(np.float16))

    x8_full = np.ascontiguousarray(
        x[:, sel8].astype(np.float32).astype(ml_dtypes.float8_e4m3).T
    )
    x16_full = np.ascontiguousarray(x[:, sel16].astype(np.float32).astype(np.float16).T)
    return w8b, w16b, x8_full, x16_full, 1.0 / S


def _run(x: np.ndarray, cores: np.ndarray, trace: bool = False, trace_cores=None):
    from concourse.bass_utils import run_bass_kernel_spmd

    w8b, w16b, x8_full, x16_full, inv_scale = _prepare(x, cores)

    in_maps = []
    for c in range(N_CORES):
        in_maps.append(
            {
                "x8": np.ascontiguousarray(x8_full[:, c * NPC : (c + 1) * NPC]),
                "x16": np.ascontiguousarray(x16_full[:, c * NPC : (c + 1) * NPC]),
                "w8": w8b,
                "w16": w16b,
            }
        )

    nc = _build_bass(inv_scale)
    kwargs = {}
    if trace_cores is not None:
        kwargs["trace_cores"] = trace_cores
    res = run_bass_kernel_spmd(
        nc, in_maps, core_ids=list(range(N_CORES)), trace=trace, **kwargs
    )

    y = np.empty((B, D), dtype=np.float32)
    for c in range(N_CORES):
        y[c * NPC : (c + 1) * NPC, :] = res.results[c]["yt"].T.astype(np.float32)
    return y, res


def kernel(x: np.ndarray, cores: np.ndarray) -> np.ndarray:
    y, _ = _run(x, cores, trace=False)
    return y
